# revision 1
# baseline (speedup 1.0000x reference)
"""GroupedQueryAttention on 8 Trainium2 NeuronCores.

Sharding: core c = 4*b + r handles batch b (of 2) and token chunk r (512
of 2048 tokens) for Q/attention/o_proj over ALL 16 heads. K/V projections
are sharded by KV group: core r computes group g=r's K/V for all T, then
one small (1 MB in / 4 MB out) AllGather across each batch's 4 cores makes
every core independent for the rest of the kernel -- no output collective.

AllGather payload kv_loc [256, 2048]: rows 0:128 K^T [hd, T]; rows
128:256 V in per-partition-major order (row 128+p, cols s*128+c holds
V[s*128+p, c]) so the consumer-side unpack is one fat [128, 2048] DMA
per group instead of 64 short-line DMAs.

Per-group token order is core-local (own chunk first): softmax + A@V are
permutation-invariant over keys as long as K and V share the order.

DMA queues: Sync carries the x^T stream, the wq stream, unpacks and
output writes; Activation carries x_own, wk/wv/biases, K/V AllGather
input writes, and the wo stream. This keeps the latency-critical
AllGather inputs and the Q-proj weight stream on independent queues.

PSUM: tag "big" = 2 bufs x [128, 1024] fp32 (scores for TWO key blocks
per buffer -> one batched exp instruction, halving Activation-engine
instruction+semaphore overhead, which paced v2's attention phase); tag
"opk" = 4 bufs x [128, 129] fp32 A@V accumulators, whose banks also host
the transient transpose outputs.

All matmuls run in fp16 (1 PE cycle/row) with fp32 PSUM accumulation.
Layouts avoid transposing the big P matrix:
  - projections produce Q^T/K^T/V^T directly (lhsT=W block, rhs=x^T block)
  - scores are computed as S^T = (K^T).T @ Q^T
  - exp(S^T) = P^T feeds A@V as lhsT directly
  - V carries an extra ones-column so the softmax denominator falls out
    of the A@V matmul for free
  - o_proj bias is added via an identity-matmul of a broadcast bias row
"""

import math
import sys

import numpy as np

sys.path.insert(0, "/opt/trn_rl_repo")

B = 2
T = 2048
D = 2048
HEADS = 16
GROUPS = 4
HD = 128  # head dim
M = HEADS // GROUPS  # heads per group = 4
SCALE = 1.0 / math.sqrt(HD)
N_CORES = 8
TCH = 512  # token chunk per core
NTCH = T // TCH  # 4
NSB = T // 128  # 16 key blocks
NKS = D // 128  # 16 contraction steps for projections
NQ = NKS // 4  # 4 quad blocks for the x stream
NNB = D // TCH  # 4 o_proj output column blocks

_COMPILED = {}


def _build():
    import concourse.bass as bass
    import concourse.mybir as mybir
    import concourse.tile as tile
    from concourse import bacc
    from concourse.masks import make_identity

    f16 = mybir.dt.float16
    f32 = mybir.dt.float32
    Exp = mybir.ActivationFunctionType.Exp

    nc = bacc.Bacc("TRN2", target_bir_lowering=False, num_devices=N_CORES)

    # x^T as (slot, quad) row-blocks of [128, 2048]; slot 0 = own chunk
    xcb_d = nc.declare_dram_parameter("xcb", [NTCH * NQ * 128, 4 * TCH], f16,
                                      isOutput=False)
    wq_d = nc.declare_dram_parameter("wq", [128, HEADS * NKS * 128], f16,
                                     isOutput=False)
    wk_d = nc.declare_dram_parameter("wk", [128, NKS * 128], f16, isOutput=False)
    wv_d = nc.declare_dram_parameter("wv", [128, NKS * 128], f16, isOutput=False)
    wo_d = nc.declare_dram_parameter("wo", [128, NNB * HEADS * TCH], f16,
                                     isOutput=False)
    bqs_d = nc.declare_dram_parameter("bqs", [128, HEADS], f32, isOutput=False)
    bks_d = nc.declare_dram_parameter("bks", [128, 1], f32, isOutput=False)
    bvs_d = nc.declare_dram_parameter("bvs", [128, 1], f32, isOutput=False)
    bob_d = nc.declare_dram_parameter("bob", [128, D], f16, isOutput=False)
    out_d = nc.declare_dram_parameter("out", [TCH, D], f32, isOutput=True)

    groups = [[0, 1, 2, 3], [4, 5, 6, 7]]

    with tile.TileContext(nc) as tc:
        with (
            tc.tile_pool(name="const", bufs=1) as const,
            tc.tile_pool(name="work", bufs=2) as work,
            tc.tile_pool(name="psum", bufs=1, space="PSUM") as psum,
            tc.tile_pool(name="dram", bufs=1, space="DRAM") as dram,
        ):
            ident = const.tile([128, 128], f16)
            make_identity(nc, ident)
            bqs = const.tile([128, HEADS], f32)
            bks = const.tile([128, 1], f32)
            bvs = const.tile([128, 1], f32)
            bob = const.tile([128, D], f16)

            wk_sb = const.tile([128, NKS, 128], f16)
            wv_sb = const.tile([128, NKS, 128], f16)
            x_own = const.tile([128, NQ, 4 * TCH], f16)
            # act-queue loads: needed by phase 1 / early phase 2
            nc.scalar.dma_start(wk_sb[:], wk_d[:])
            nc.scalar.dma_start(wv_sb[:], wv_d[:])
            nc.scalar.dma_start(bks[:], bks_d[:])
            nc.scalar.dma_start(bvs[:], bvs_d[:])
            nc.scalar.dma_start(bqs[:], bqs_d[:])
            for q in range(NQ):
                nc.scalar.dma_start(x_own[:, q, :], xcb_d[q * 128 : (q + 1) * 128, :])

            kt = const.tile([128, GROUPS, T], f16)  # gathered K^T
            v_sb = const.tile([128, GROUPS, NSB, 132], f16)  # gathered V + ones
            qt = const.tile([128, HEADS, TCH], f16)  # own-chunk Q^T
            at = const.tile([128, HEADS, TCH], f16)  # own-chunk A^T
            nc.vector.memset(v_sb[:, :, :, 128:129], 1.0)

            # AllGather payload: rows 0:128 K^T, rows 128:256 V p-major
            kv_loc = dram.tile([256, T], f16, tag="kvl", name="kv_loc")
            kv_g = dram.tile([GROUPS * 256, T], f16, tag="kvg", name="kv_g")

            # wq stream: 6-deep, alternating across both HWDGE queues so
            # Q-proj is compute-paced, not DMA-paced
            NWQB = 6
            wq_tiles = {}

            def issue_wq(h, eng=None):
                wqh = work.tile([128, NKS * 128], f16, tag="wq", bufs=NWQB,
                                name="wqh", uniquify=True)
                if eng is None:
                    eng = nc.scalar if h % 2 else nc.sync
                eng.dma_start(
                    wqh[:], wq_d[:, h * NKS * 128 : (h + 1) * NKS * 128])
                wq_tiles[h] = wqh

            issue_wq(0, nc.scalar)
            issue_wq(1, nc.scalar)
            issue_wq(2, nc.scalar)

            # ---- phase 1: K/V projection for own group, all T ----
            # Half the K/V AllGather-input writes go on the sync queue, but
            # deferred: their triggers are emitted after the NEXT chunk's
            # stream-quad triggers so they never head-of-line block the
            # stream (in-order queue).
            pending_w = []

            def flush_w(n=99):
                for _ in range(min(n, len(pending_w))):
                    dst, srcap = pending_w.pop(0)
                    nc.sync.dma_start(dst, srcap)

            for tc2 in [1, 2, 3, 0]:
                kacc = psum.tile([128, TCH], f32, tag="opk", bufs=4, name="kacc")
                vacc = psum.tile([128, TCH], f32, tag="opk", bufs=4, name="vacc")
                for q in range(NQ):
                    if tc2 == 0:
                        x4 = x_own[:, q, :]
                    else:
                        x4t = work.tile([128, 4 * TCH], f16, tag="xs", bufs=3,
                                        name="x4t")
                        nc.sync.dma_start(
                            x4t[:],
                            xcb_d[(tc2 * NQ + q) * 128 : (tc2 * NQ + q + 1) * 128, :],
                        )
                        flush_w(1)
                        x4 = x4t[:]
                    for k2 in range(4):
                        ks = q * 4 + k2
                        xb = x4[:, k2 * TCH : (k2 + 1) * TCH]
                        nc.tensor.matmul(
                            kacc[:], wk_sb[:, ks, :], xb,
                            start=(ks == 0), stop=(ks == NKS - 1),
                        )
                        nc.tensor.matmul(
                            vacc[:], wv_sb[:, ks, :], xb,
                            start=(ks == 0), stop=(ks == NKS - 1),
                        )
                ktc = work.tile([128, TCH], f16, tag="ktc", bufs=2, name="ktc")
                nc.vector.tensor_scalar_add(ktc[:], kacc[:], bks[:, 0:1])
                kdst = kv_loc[0:128, tc2 * TCH : (tc2 + 1) * TCH]
                if tc2 == 0:
                    nc.scalar.dma_start(kdst, ktc[:])
                else:
                    pending_w.append((kdst, ktc[:]))
                vtc = work.tile([128, TCH], f16, tag="vtc", bufs=2, name="vtc")
                nc.vector.tensor_scalar_add(vtc[:], vacc[:], bvs[:, 0:1])
                for sb in range(4):
                    sbg = tc2 * 4 + sb
                    tp = psum.tile([128, 128], f16, tag="big", bufs=2, name="tp")
                    nc.tensor.transpose(tp[:], vtc[:, sb * 128 : (sb + 1) * 128],
                                        ident[:])
                    vn = work.tile([128, 128], f16, tag="vn", bufs=4, name="vn")
                    nc.vector.tensor_copy(vn[:], tp[:])
                    # V block, p-major: row 128+p, cols sbg*128..+128
                    vdst = kv_loc[128:256, sbg * 128 : (sbg + 1) * 128]
                    if tc2 == 0 or sb >= 2:
                        nc.scalar.dma_start(vdst, vn[:])
                    else:
                        pending_w.append((vdst, vn[:]))
            flush_w()

            nc.gpsimd.collective_compute(
                "AllGather", mybir.AluOpType.bypass, replica_groups=groups,
                ins=[kv_loc[:]], outs=[kv_g[:]],
            )
            issue_wq(3, nc.sync)
            issue_wq(4, nc.sync)
            issue_wq(5, nc.sync)


            # ---- phase 2: Q projection (own chunk, all heads); overlaps AG ----
            for h in range(HEADS):
                wqh = wq_tiles[h]
                qacc = psum.tile([128, TCH], f32, tag="opk", bufs=4, name="qacc")
                for ks in range(NKS):
                    nc.tensor.matmul(
                        qacc[:], wqh[:, ks * 128 : (ks + 1) * 128],
                        x_own[:, ks // 4, (ks % 4) * TCH : (ks % 4 + 1) * TCH],
                        start=(ks == 0), stop=(ks == NKS - 1),
                    )
                nc.vector.tensor_scalar(
                    qt[:, h, :], qacc[:], SCALE, bqs[:, h : h + 1],
                    op0=mybir.AluOpType.mult, op1=mybir.AluOpType.add,
                )
                if h + NWQB < HEADS:
                    issue_wq(h + NWQB)  # alternates queues by parity

            # unpack gathered K^T / V into SBUF, split across both queues
            # (triggers wait on the AllGather semaphore, then run immediately)
            for g in range(GROUPS):
                nc.sync.dma_start(kt[:, g, :], kv_g[g * 256 : g * 256 + 128, :])
                nc.scalar.dma_start(
                    v_sb[:, g, :, 0:128], kv_g[g * 256 + 128 : (g + 1) * 256, :]
                )

            # ---- phase 3: attention for own chunk, all heads ----
            # Normalize/transpose of head h-1 is issued after head h's first
            # score pair so the PE never stalls on the DVE normalize chain.
            pending = []

            def normalize(h, opks):
                for tb in range(4):
                    opk = opks[tb]
                    rcp = work.tile([128, 1], f32, tag="rcp", bufs=2, name="rcp")
                    nc.vector.reciprocal(rcp[:], opk[:, 128:129])
                    o_sb = work.tile([128, 128], f16, tag="osb", bufs=2, name="osb")
                    nc.vector.tensor_scalar_mul(o_sb[:], opk[:, 0:128], rcp[:])
                    tp = psum.tile([128, 128], f16, tag="big", bufs=2, name="tpo")
                    nc.tensor.transpose(tp[:], o_sb[:], ident[:])
                    nc.vector.tensor_copy(at[:, h, tb * 128 : (tb + 1) * 128],
                                          tp[:])

            for g in range(GROUPS):
                for hh in range(M):
                    h = g * M + hh
                    opks = [
                        psum.tile([128, 129], f32, tag="opk", bufs=4, name=f"opk{i}")
                        for i in range(4)
                    ]
                    for sp in range(NSB // 2):
                        sps2 = psum.tile([128, 2 * TCH], f32, tag="big", bufs=2,
                                         name="sps2")
                        for j in range(2):
                            s = sp * 2 + j
                            nc.tensor.matmul(
                                sps2[:, j * TCH : (j + 1) * TCH],
                                kt[:, g, s * 128 : (s + 1) * 128], qt[:, h, :],
                                start=True, stop=True,
                            )
                        if sp == 0 and pending:
                            normalize(*pending.pop())
                        p2 = work.tile([128, 2 * TCH], f16, tag="p", bufs=4,
                                       name="p2")
                        nc.scalar.activation(p2[:], sps2[:], Exp)
                        for j in range(2):
                            s = sp * 2 + j
                            for tb in range(4):
                                nc.tensor.matmul(
                                    opks[tb][:, 0:129],
                                    p2[:, j * TCH + tb * 128 : j * TCH + (tb + 1) * 128],
                                    v_sb[:, g, s, 0:129],
                                    start=(s == 0), stop=(s == NSB - 1),
                                )
                    pending.append((h, opks))
            normalize(*pending.pop())

            # ---- phase 4: o_proj for own chunk, full D ----
            nc.scalar.dma_start(bob[:], bob_d[:])
            for nb in range(NNB):
                wob = work.tile([128, HEADS * TCH], f16, tag="wo", bufs=2, name="wob")
                nc.scalar.dma_start(
                    wob[:], wo_d[:, nb * HEADS * TCH : (nb + 1) * HEADS * TCH]
                )
                for tb in range(4):
                    pp = psum.tile([128, TCH], f32, tag="big", bufs=2, name="pp")
                    # bias row via identity matmul: out[m, n] += bob[m, n]
                    nc.tensor.matmul(
                        pp[:], ident[:], bob[:, nb * TCH : (nb + 1) * TCH],
                        start=True, stop=False,
                    )
                    for h in range(HEADS):
                        nc.tensor.matmul(
                            pp[:],
                            at[:, h, tb * 128 : (tb + 1) * 128],
                            wob[:, h * TCH : (h + 1) * TCH],
                            start=False, stop=(h == HEADS - 1),
                        )
                    ob = work.tile([128, TCH], f32, tag="ob", bufs=3, name="ob")
                    nc.vector.tensor_copy(ob[:], pp[:])
                    nc.sync.dma_start(
                        out_d[tb * 128 : (tb + 1) * 128, nb * TCH : (nb + 1) * TCH],
                        ob[:],
                    )

    nc.compile()
    return nc


def _get_nc():
    if "nc" not in _COMPILED:
        _COMPILED["nc"] = _build()
    return _COMPILED["nc"]


def kernel(x, Wq, bq, Wk, bk, Wv, bv, Wo, bo):
    from concourse.bass_utils import run_bass_kernel_spmd

    x = np.asarray(x, np.float32)
    Wq = np.asarray(Wq, np.float32)
    Wk = np.asarray(Wk, np.float32)
    Wv = np.asarray(Wv, np.float32)
    Wo = np.asarray(Wo, np.float32)
    bq = np.asarray(bq, np.float32)
    bk = np.asarray(bk, np.float32)
    bv = np.asarray(bv, np.float32)
    bo = np.asarray(bo, np.float32)

    nc = _get_nc()

    # shared across cores
    wq_h = np.ascontiguousarray(
        Wq.reshape(NKS, 128, HEADS, 128).transpose(1, 2, 0, 3).reshape(128, -1)
    ).astype(np.float16)
    wo_h = np.ascontiguousarray(
        Wo.reshape(HEADS, 128, NNB, TCH).transpose(1, 2, 0, 3).reshape(128, -1)
    ).astype(np.float16)
    bqs_h = np.ascontiguousarray((bq * SCALE).reshape(HEADS, 128).T)
    bob_h = np.ascontiguousarray(np.broadcast_to(bo.astype(np.float16), (128, D)))
    # x^T per batch, pre-blocked into (chunk, quad) [128, 2048] row-blocks
    xq16 = []
    for b in range(B):
        xTb = x[b].T.astype(np.float16)  # [D, T]
        blocks = xTb.reshape(NKS, 128, NTCH, TCH).transpose(2, 0, 1, 3)
        # [chunk, ks, 128, TCH] -> quads: [chunk, quad, 128, 4*TCH]
        blocks = blocks.reshape(NTCH, NQ, 4, 128, TCH).transpose(0, 1, 3, 2, 4)
        xq16.append(np.ascontiguousarray(blocks.reshape(NTCH, NQ * 128, 4 * TCH)))

    wk_g, wv_g, bks_g, bvs_g = [], [], [], []
    for g in range(GROUPS):
        wk_g.append(
            np.ascontiguousarray(
                Wk[:, g * HD : (g + 1) * HD].reshape(NKS, 128, HD)
                .transpose(1, 0, 2).reshape(128, -1)
            ).astype(np.float16)
        )
        wv_g.append(
            np.ascontiguousarray(
                Wv[:, g * HD : (g + 1) * HD].reshape(NKS, 128, HD)
                .transpose(1, 0, 2).reshape(128, -1)
            ).astype(np.float16)
        )
        bks_g.append(np.ascontiguousarray(bk[g * HD : (g + 1) * HD].reshape(1, HD).T))
        bvs_g.append(np.ascontiguousarray(bv[g * HD : (g + 1) * HD].reshape(1, HD).T))

    in_maps = []
    for c in range(N_CORES):
        b, r = c // 4, c % 4
        order = [r] + [i for i in range(NTCH) if i != r]
        xcb = np.concatenate([xq16[b][s] for s in order], axis=0)
        in_maps.append(
            {
                "xcb": np.ascontiguousarray(xcb),
                "wq": wq_h,
                "wk": wk_g[r],
                "wv": wv_g[r],
                "wo": wo_h,
                "bqs": bqs_h,
                "bks": bks_g[r],
                "bvs": bvs_g[r],
                "bob": bob_h,
            }
        )

    res = run_bass_kernel_spmd(nc, in_maps, list(range(N_CORES)))
    _COMPILED["last_res"] = res

    out = np.empty((B, T, D), np.float32)
    for b in range(B):
        for r in range(NTCH):
            out[b, r * TCH : (r + 1) * TCH, :] = res.results[4 * b + r]["out"]
    return out



# revision 5
# speedup vs baseline: 1.0603x; 1.0603x over previous
"""GroupedQueryAttention on 8 Trainium2 NeuronCores (v3).

Sharding: core c = 4*b + r handles batch b (of 2) and token chunk r (512
of 2048 tokens) for Q/attention/o_proj over ALL 16 heads. K/V projections
are sharded by KV group: core r computes group g=r's K/V for all T, then
chunk-split AllGathers across each batch's 4 cores make every core
independent for the rest of the kernel -- no output collective.

v3 deltas over the 492us baseline:
  - phase 1 runs chunks in payload order [own,1,2,3] and pushes each
    chunk's K/V payload immediately; the AllGather is split in
    AG_SPLIT pieces issued as soon as their chunks are done, so the
    collective overlaps phase 1 + Q-proj instead of trailing them.
  - the 8 MB wq stream lives on the scalar HWDGE queue (paced 1-2 tiles
    per phase-1 chunk, 7-deep) so Q-proj never starves: the baseline
    lost ~50us to wq waiting behind x/unpack traffic on sync.
  - every PE transpose (16 V blocks + 64 attention outputs) is replaced
    by an XBAR dma_start_transpose ([128,512] -> [128,4,128] blocked
    transpose, ~0.5us on a DMA queue), freeing ~14us of PE and the DVE
    copies that drained them.
  - o_proj is h-outer (each at-block stationary feeds 4 matmuls), bias
    is a DVE add fused into the PSUM drain (identity-matmul bias gone),
    and out is written fp16 (host upcasts).
  - Wo lives in one 8 MB SBUF buffer whose first quarter aliases x_own
    (dead after Q-proj); Wo head-tiles 4..15 stream during attention,
    0..3 after Q-proj reads retire. Scalar queue carries no DMA during
    attention so exp (the 138us ACT-bound softmax) never waits.
  - PSUM: tag "big" = 3 x [128,1024] (scores 3-deep, o_proj nb-pairs),
    tag "half" = 2 x [128,512]-sized slots holding K/V/Q accumulators
    and the paired A@V accumulators [128,258] (2 x (128 out + 1 denom)).

All matmuls fp16 (1 PE cycle/row) with fp32 PSUM accumulation. Layouts
avoid transposing the big P matrix: projections produce Q^T/K^T/V^T
directly; scores are S^T = (K^T block).T @ Q^T; exp(S^T) = P^T feeds
A@V as the stationary; V carries a ones-column so the softmax
denominator falls out of the A@V matmul for free.
"""

import math
import sys

import numpy as np

sys.path.insert(0, "/opt/trn_rl_repo")

B = 2
T = 2048
D = 2048
HEADS = 16
GROUPS = 4
HD = 128  # head dim
M = HEADS // GROUPS  # heads per group = 4
SCALE = 1.0 / math.sqrt(HD)
N_CORES = 8
TCH = 512  # token chunk per core
NTCH = T // TCH  # 4
NSB = T // 128  # 16 key blocks
NKS = D // 128  # 16 contraction steps for projections
NQ = NKS // 4  # 4 quad blocks for the x stream
NNB = D // TCH  # 4 o_proj output column blocks

NWQB = 7  # wq stream depth
AG_SPLIT = 2  # number of chunk-split AllGathers
CPA = NTCH // AG_SPLIT  # chunks per AllGather

_COMPILED = {}


def _build():
    import concourse.bass as bass
    import concourse.mybir as mybir
    import concourse.tile as tile
    from concourse import bacc

    f16 = mybir.dt.float16
    f32 = mybir.dt.float32
    Exp = mybir.ActivationFunctionType.Exp
    Mult = mybir.AluOpType.mult
    Add = mybir.AluOpType.add

    nc = bacc.Bacc("TRN2", target_bir_lowering=False, num_devices=N_CORES)

    # x^T as (slot, quad) row-blocks of [128, 2048]; slot 0 = own chunk
    xcb_d = nc.declare_dram_parameter("xcb", [NTCH * NQ * 128, 4 * TCH], f16,
                                      isOutput=False)
    wq_d = nc.declare_dram_parameter("wq", [128, HEADS * NKS * 128], f16,
                                     isOutput=False)
    wk_d = nc.declare_dram_parameter("wk", [128, NKS * 128], f16, isOutput=False)
    wv_d = nc.declare_dram_parameter("wv", [128, NKS * 128], f16, isOutput=False)
    # by-head layout: [128, h, nb, 512]
    wo_d = nc.declare_dram_parameter("wo", [128, HEADS * NNB * TCH], f16,
                                     isOutput=False)
    bqs_d = nc.declare_dram_parameter("bqs", [128, HEADS], f32, isOutput=False)
    bks_d = nc.declare_dram_parameter("bks", [128, 1], f32, isOutput=False)
    bvs_d = nc.declare_dram_parameter("bvs", [128, 1], f32, isOutput=False)
    bob_d = nc.declare_dram_parameter("bob", [128, D], f16, isOutput=False)
    out_d = nc.declare_dram_parameter("out", [TCH, D], f16, isOutput=True)

    groups = [[0, 1, 2, 3], [4, 5, 6, 7]]

    with tile.TileContext(nc) as tc:
        with (
            tc.tile_pool(name="const", bufs=1) as const,
            tc.tile_pool(name="work", bufs=2) as work,
            tc.tile_pool(name="psum", bufs=1, space="PSUM") as psum,
            tc.tile_pool(name="dram", bufs=1, space="DRAM") as dram,
        ):
            bqs = const.tile([128, HEADS], f32)
            bks = const.tile([128, 1], f32)
            bvs = const.tile([128, 1], f32)
            bob = const.tile([128, D], f16)

            wk_sb = const.tile([128, NKS, 128], f16)
            wv_sb = const.tile([128, NKS, 128], f16)
            # Wo by head; first NQ head-slots double as x_own (dead by o_proj)
            wo_sb = const.tile([128, HEADS, NNB * TCH], f16)
            x_own = wo_sb[:, 0:NQ, :]

            kt = const.tile([128, GROUPS, T], f16)  # gathered K^T
            v_sb = const.tile([128, GROUPS, NSB, 132], f16)  # gathered V + ones
            qt = const.tile([128, HEADS, TCH], f16)  # own-chunk Q^T
            at = const.tile([128, HEADS, 4, 128], f16)  # own-chunk A^T
            nc.vector.memset(v_sb[:, :, :, 128:129], 1.0)

            # initial loads, quad-interleaved so chunk 0 can start early
            for q in range(NQ):
                nc.scalar.dma_start(wk_sb[:, q * 4 : (q + 1) * 4, :],
                                    wk_d[:, q * 512 : (q + 1) * 512])
                nc.scalar.dma_start(wv_sb[:, q * 4 : (q + 1) * 4, :],
                                    wv_d[:, q * 512 : (q + 1) * 512])
                nc.scalar.dma_start(x_own[:, q, :],
                                    xcb_d[q * 128 : (q + 1) * 128, :])
            nc.scalar.dma_start(bks[:], bks_d[:])
            nc.scalar.dma_start(bvs[:], bvs_d[:])
            nc.scalar.dma_start(bqs[:], bqs_d[:])

            # wq stream on the scalar HWDGE queue
            wq_tiles = {}

            def issue_wq(h):
                if h >= HEADS or h in wq_tiles:
                    return
                wqh = work.tile([128, NKS * 128], f16, tag="wq", bufs=NWQB,
                                name="wqh", uniquify=True)
                nc.scalar.dma_start(
                    wqh[:], wq_d[:, h * NKS * 128 : (h + 1) * NKS * 128])
                wq_tiles[h] = wqh

            issue_wq(0)
            issue_wq(1)

            # ---- phase 1: K/V projection for own group, all T ----
            kvl = {}
            kvg = {}
            for a in range(AG_SPLIT):
                kvl[a] = dram.tile([256, CPA * TCH], f16, tag=f"kvl{a}",
                                   name=f"kvl{a}")
                kvg[a] = dram.tile([4 * 256, CPA * TCH], f16, tag=f"kvg{a}",
                                   name=f"kvg{a}")

            for c in range(NTCH):
                kacc = psum.tile([128, TCH], f32, tag="half", bufs=2, name="kacc")
                vacc = psum.tile([128, TCH], f32, tag="half", bufs=2, name="vacc")
                for q in range(NQ):
                    if c == 0:
                        x4 = x_own[:, q, :]
                    else:
                        x4t = work.tile([128, 4 * TCH], f16, tag="xs", bufs=3,
                                        name="x4t")
                        nc.sync.dma_start(
                            x4t[:],
                            xcb_d[(c * NQ + q) * 128 : (c * NQ + q + 1) * 128, :],
                        )
                        x4 = x4t[:]
                    for k2 in range(4):
                        ks = q * 4 + k2
                        xb = x4[:, k2 * TCH : (k2 + 1) * TCH]
                        nc.tensor.matmul(
                            kacc[:], wk_sb[:, ks, :], xb,
                            start=(ks == 0), stop=(ks == NKS - 1),
                        )
                        nc.tensor.matmul(
                            vacc[:], wv_sb[:, ks, :], xb,
                            start=(ks == 0), stop=(ks == NKS - 1),
                        )
                a, cc = c // CPA, c % CPA
                ktc = work.tile([128, TCH], f16, tag="ktc", bufs=2, name="ktc")
                nc.vector.tensor_scalar_add(ktc[:], kacc[:], bks[:, 0:1])
                nc.scalar.dma_start(kvl[a][0:128, cc * TCH : (cc + 1) * TCH],
                                    ktc[:])
                vtc = work.tile([128, TCH], f16, tag="vtc", bufs=2, name="vtc")
                nc.vector.tensor_scalar_add(vtc[:], vacc[:], bvs[:, 0:1])
                # blocked XBAR transpose: vT[:, b, :] = vtc[:, b*128:+128].T
                vT = work.tile([128, 4, 128], f16, tag="vt", bufs=2, name="vT")
                nc.scalar.dma_start_transpose(vT[:], vtc[:])
                nc.scalar.dma_start(kvl[a][128:256, cc * TCH : (cc + 1) * TCH],
                                    vT[:])
                # cap at tile NWQB-1: deeper tiles would wait on Q-proj
                # consumption and head-of-line block the scalar queue
                issue_wq(min(2 + 2 * c, NWQB - 1))
                issue_wq(min(3 + 2 * c, NWQB - 1))
                if cc == CPA - 1:
                    nc.gpsimd.collective_compute(
                        "AllGather", mybir.AluOpType.bypass,
                        replica_groups=groups,
                        ins=[kvl[a][:]], outs=[kvg[a][:]],
                    )

            # ---- phase 2: Q projection (own chunk, all heads); overlaps AG ----
            for h in range(HEADS):
                wqh = wq_tiles[h]
                qacc = psum.tile([128, TCH], f32, tag="half", bufs=2, name="qacc")
                for ks in range(NKS):
                    nc.tensor.matmul(
                        qacc[:], wqh[:, ks * 128 : (ks + 1) * 128],
                        x_own[:, ks // 4, (ks % 4) * TCH : (ks % 4 + 1) * TCH],
                        start=(ks == 0), stop=(ks == NKS - 1),
                    )
                nc.vector.tensor_scalar(
                    qt[:, h, :], qacc[:], SCALE, bqs[:, h : h + 1],
                    op0=Mult, op1=Add,
                )
                issue_wq(h + NWQB)

            nc.scalar.dma_start(bob[:], bob_d[:])

            # unpack gathered K^T / V into SBUF; groups 0/1 on sync run as
            # soon as each AG lands, groups 2/3 ride scalar behind the wq
            # stream (resolved well before their attention turn)
            for g in range(GROUPS):
                eng = nc.sync if g < 2 else nc.scalar
                for a in range(AG_SPLIT):
                    eng.dma_start(
                        kt[:, g, a * CPA * TCH : (a + 1) * CPA * TCH],
                        kvg[a][g * 256 : g * 256 + 128, :],
                    )
                    eng.dma_start(
                        v_sb[:, g, a * CPA * 4 : (a + 1) * CPA * 4, 0:128],
                        kvg[a][g * 256 + 128 : (g + 1) * 256, :],
                    )

            # ---- phase 3: attention for own chunk, all heads ----
            # Wo head-tiles stream on gpsimd during attention (one per head
            # slot, heads 0..3 additionally wait out Q-proj's x_own reads)
            wo_order = list(range(NQ, HEADS)) + list(range(NQ))
            for g in range(GROUPS):
                for hh in range(M):
                    h = g * M + hh
                    nc.gpsimd.dma_start(
                        wo_sb[:, wo_order[h], :],
                        wo_d[:, wo_order[h] * NNB * TCH
                             : (wo_order[h] + 1) * NNB * TCH],
                    )
                    opk01 = psum.tile([128, 258], f32, tag="half", bufs=2,
                                      name="opk01")
                    opk23 = psum.tile([128, 258], f32, tag="half", bufs=2,
                                      name="opk23")
                    opks = [(opk01, 0), (opk01, 129), (opk23, 0), (opk23, 129)]
                    for sp in range(NSB // 2):
                        sps2 = psum.tile([128, 2 * TCH], f32, tag="big", bufs=3,
                                         name="sps2")
                        for j in range(2):
                            s = sp * 2 + j
                            nc.tensor.matmul(
                                sps2[:, j * TCH : (j + 1) * TCH],
                                kt[:, g, s * 128 : (s + 1) * 128], qt[:, h, :],
                                start=True, stop=True,
                            )
                        p2 = work.tile([128, 2 * TCH], f16, tag="p", bufs=4,
                                       name="p2")
                        nc.scalar.activation(p2[:], sps2[:], Exp)
                        for j in range(2):
                            s = sp * 2 + j
                            for tb in range(4):
                                opk, off = opks[tb]
                                # start=True clears the WHOLE PSUM bank's
                                # has_written bits, so only the first group
                                # in each shared bank may assert it; the
                                # off=129 group's s=0 matmul writes fresh
                                # (per-element has_written=0) with start=False
                                nc.tensor.matmul(
                                    opk[:, off : off + 129],
                                    p2[:, j * TCH + tb * 128
                                       : j * TCH + (tb + 1) * 128],
                                    v_sb[:, g, s, 0:129],
                                    start=(s == 0 and off == 0),
                                    stop=(s == NSB - 1),
                                    skip_group_check=(off != 0),
                                )
                    o_sb = work.tile([128, TCH], f16, tag="osb", bufs=2,
                                     name="o_sb")
                    for tb in range(4):
                        opk, off = opks[tb]
                        rcp = work.tile([128, 1], f32, tag="rcp", bufs=4,
                                        name="rcp")
                        nc.vector.reciprocal(rcp[:], opk[:, off + 128 : off + 129])
                        nc.vector.tensor_scalar_mul(
                            o_sb[:, tb * 128 : (tb + 1) * 128],
                            opk[:, off : off + 128], rcp[:])
                    # at[:, h, tb, :] = o_sb[:, tb*128:+128].T via XBAR
                    nc.sync.dma_start_transpose(at[:, h], o_sb[:])

            # ---- phase 4: o_proj for own chunk, full D ----
            for tb in range(4):
                pp01 = psum.tile([128, 2 * TCH], f32, tag="big", bufs=3,
                                 name="pp01")
                pp23 = psum.tile([128, 2 * TCH], f32, tag="big", bufs=3,
                                 name="pp23")
                pps = [(pp01, 0), (pp01, TCH), (pp23, 0), (pp23, TCH)]
                for h in range(HEADS):
                    for nb in range(NNB):
                        pp, off = pps[nb]
                        nc.tensor.matmul(
                            pp[:, off : off + TCH],
                            at[:, h, tb, :],
                            wo_sb[:, h, nb * TCH : (nb + 1) * TCH],
                            start=(h == 0), stop=(h == HEADS - 1),
                        )
                for nb in range(NNB):
                    pp, off = pps[nb]
                    ob = work.tile([128, TCH], f16, tag="ob", bufs=4, name="ob")
                    nc.vector.scalar_tensor_tensor(
                        ob[:], pp[:, off : off + TCH], 1.0,
                        bob[:, nb * TCH : (nb + 1) * TCH],
                        op0=Mult, op1=Add,
                    )
                    nc.sync.dma_start(
                        out_d[tb * 128 : (tb + 1) * 128, nb * TCH : (nb + 1) * TCH],
                        ob[:],
                    )

    nc.compile()
    return nc


def _get_nc():
    if "nc" not in _COMPILED:
        _COMPILED["nc"] = _build()
    return _COMPILED["nc"]


def kernel(x, Wq, bq, Wk, bk, Wv, bv, Wo, bo):
    from concourse.bass_utils import run_bass_kernel_spmd

    x = np.asarray(x, np.float32)
    Wq = np.asarray(Wq, np.float32)
    Wk = np.asarray(Wk, np.float32)
    Wv = np.asarray(Wv, np.float32)
    Wo = np.asarray(Wo, np.float32)
    bq = np.asarray(bq, np.float32)
    bk = np.asarray(bk, np.float32)
    bv = np.asarray(bv, np.float32)
    bo = np.asarray(bo, np.float32)

    nc = _get_nc()

    # shared across cores
    wq_h = np.ascontiguousarray(
        Wq.reshape(NKS, 128, HEADS, 128).transpose(1, 2, 0, 3).reshape(128, -1)
    ).astype(np.float16)
    # by-head o_proj layout [128, h, nb, 512]
    wo_h = np.ascontiguousarray(
        Wo.reshape(HEADS, 128, NNB, TCH).transpose(1, 0, 2, 3).reshape(128, -1)
    ).astype(np.float16)
    bqs_h = np.ascontiguousarray((bq * SCALE).reshape(HEADS, 128).T)
    bob_h = np.ascontiguousarray(np.broadcast_to(bo.astype(np.float16), (128, D)))
    # x^T per batch, pre-blocked into (chunk, quad) [128, 2048] row-blocks
    xq16 = []
    for b in range(B):
        xTb = x[b].T.astype(np.float16)  # [D, T]
        blocks = xTb.reshape(NKS, 128, NTCH, TCH).transpose(2, 0, 1, 3)
        # [chunk, ks, 128, TCH] -> quads: [chunk, quad, 128, 4*TCH]
        blocks = blocks.reshape(NTCH, NQ, 4, 128, TCH).transpose(0, 1, 3, 2, 4)
        xq16.append(np.ascontiguousarray(blocks.reshape(NTCH, NQ * 128, 4 * TCH)))

    wk_g, wv_g, bks_g, bvs_g = [], [], [], []
    for g in range(GROUPS):
        wk_g.append(
            np.ascontiguousarray(
                Wk[:, g * HD : (g + 1) * HD].reshape(NKS, 128, HD)
                .transpose(1, 0, 2).reshape(128, -1)
            ).astype(np.float16)
        )
        wv_g.append(
            np.ascontiguousarray(
                Wv[:, g * HD : (g + 1) * HD].reshape(NKS, 128, HD)
                .transpose(1, 0, 2).reshape(128, -1)
            ).astype(np.float16)
        )
        bks_g.append(np.ascontiguousarray(bk[g * HD : (g + 1) * HD].reshape(1, HD).T))
        bvs_g.append(np.ascontiguousarray(bv[g * HD : (g + 1) * HD].reshape(1, HD).T))

    in_maps = []
    for c in range(N_CORES):
        b, r = c // 4, c % 4
        order = [r] + [i for i in range(NTCH) if i != r]
        xcb = np.concatenate([xq16[b][s] for s in order], axis=0)
        in_maps.append(
            {
                "xcb": np.ascontiguousarray(xcb),
                "wq": wq_h,
                "wk": wk_g[r],
                "wv": wv_g[r],
                "wo": wo_h,
                "bqs": bqs_h,
                "bks": bks_g[r],
                "bvs": bvs_g[r],
                "bob": bob_h,
            }
        )

    res = run_bass_kernel_spmd(nc, in_maps, list(range(N_CORES)))
    _COMPILED["last_res"] = res

    out = np.empty((B, T, D), np.float32)
    for b in range(B):
        for r in range(NTCH):
            out[b, r * TCH : (r + 1) * TCH, :] = (
                res.results[4 * b + r]["out"].astype(np.float32)
            )
    return out


# revision 11
# speedup vs baseline: 1.1323x; 1.0679x over previous
"""GroupedQueryAttention on 8 Trainium2 NeuronCores (v3).

Sharding: core c = 4*b + r handles batch b (of 2) and token chunk r (512
of 2048 tokens) for Q/attention/o_proj over ALL 16 heads. K/V projections
are sharded by KV group: core r computes group g=r's K/V for all T, then
chunk-split AllGathers across each batch's 4 cores make every core
independent for the rest of the kernel -- no output collective.

v3 deltas over the 492us baseline:
  - phase 1 runs chunks in payload order [own,1,2,3] and pushes each
    chunk's K/V payload immediately; the AllGather is split in
    AG_SPLIT pieces issued as soon as their chunks are done, so the
    collective overlaps phase 1 + Q-proj instead of trailing them.
  - the 8 MB wq stream lives on the scalar HWDGE queue (paced 1-2 tiles
    per phase-1 chunk, 7-deep) so Q-proj never starves: the baseline
    lost ~50us to wq waiting behind x/unpack traffic on sync.
  - every PE transpose (16 V blocks + 64 attention outputs) is replaced
    by an XBAR dma_start_transpose ([128,512] -> [128,4,128] blocked
    transpose, ~0.5us on a DMA queue), freeing ~14us of PE and the DVE
    copies that drained them.
  - o_proj is h-outer (each at-block stationary feeds 4 matmuls), bias
    is a DVE add fused into the PSUM drain (identity-matmul bias gone),
    and out is written fp16 (host upcasts).
  - Wo lives in one 8 MB SBUF buffer whose first quarter aliases x_own
    (dead after Q-proj); Wo head-tiles 4..15 stream during attention,
    0..3 after Q-proj reads retire. Scalar queue carries no DMA during
    attention so exp (the 138us ACT-bound softmax) never waits.
  - PSUM: tag "big" = 3 x [128,1024] (scores 3-deep, o_proj nb-pairs),
    tag "half" = 2 x [128,512]-sized slots holding K/V/Q accumulators
    and the paired A@V accumulators [128,258] (2 x (128 out + 1 denom)).

All matmuls fp16 (1 PE cycle/row) with fp32 PSUM accumulation. Layouts
avoid transposing the big P matrix: projections produce Q^T/K^T/V^T
directly; scores are S^T = (K^T block).T @ Q^T; exp(S^T) = P^T feeds
A@V as the stationary; V carries a ones-column so the softmax
denominator falls out of the A@V matmul for free.
"""

import math
import sys

import numpy as np

sys.path.insert(0, "/opt/trn_rl_repo")

B = 2
T = 2048
D = 2048
HEADS = 16
GROUPS = 4
HD = 128  # head dim
M = HEADS // GROUPS  # heads per group = 4
SCALE = 1.0 / math.sqrt(HD)
N_CORES = 8
TCH = 512  # token chunk per core
NTCH = T // TCH  # 4
NSB = T // 128  # 16 key blocks
NKS = D // 128  # 16 contraction steps for projections
NQ = NKS // 4  # 4 quad blocks for the x stream
NNB = D // TCH  # 4 o_proj output column blocks

NWQB = 6  # wq stream depth
AG_SPLIT = 2  # number of chunk-split AllGathers
CPA = NTCH // AG_SPLIT  # chunks per AllGather

_COMPILED = {}


def _build():
    import concourse.bass as bass
    import concourse.mybir as mybir
    import concourse.tile as tile
    from concourse import bacc
    from concourse.masks import make_identity

    f16 = mybir.dt.float16
    f32 = mybir.dt.float32
    Exp = mybir.ActivationFunctionType.Exp
    Mult = mybir.AluOpType.mult
    Add = mybir.AluOpType.add

    nc = bacc.Bacc("TRN2", target_bir_lowering=False, num_devices=N_CORES)

    # x^T as (slot, quad) row-blocks of [128, 2048]; slot 0 = own chunk
    xcb_d = nc.declare_dram_parameter("xcb", [NTCH * NQ * 128, 4 * TCH], f16,
                                      isOutput=False)
    wq_d = nc.declare_dram_parameter("wq", [128, HEADS * NKS * 128], f16,
                                     isOutput=False)
    wk_d = nc.declare_dram_parameter("wk", [128, NKS * 128], f16, isOutput=False)
    wv_d = nc.declare_dram_parameter("wv", [128, NKS * 128], f16, isOutput=False)
    # by-head layout: [128, h, nb, 512]
    wo_d = nc.declare_dram_parameter("wo", [128, HEADS * NNB * TCH], f16,
                                     isOutput=False)
    bqs_d = nc.declare_dram_parameter("bqs", [128, HEADS], f32, isOutput=False)
    bks_d = nc.declare_dram_parameter("bks", [128, 1], f32, isOutput=False)
    bvs_d = nc.declare_dram_parameter("bvs", [128, 1], f32, isOutput=False)
    bob_d = nc.declare_dram_parameter("bob", [128, D], f16, isOutput=False)
    out_d = nc.declare_dram_parameter("out", [TCH, D], f16, isOutput=True)

    groups = [[0, 1, 2, 3], [4, 5, 6, 7]]

    with tile.TileContext(nc) as tc:
        with (
            tc.tile_pool(name="const", bufs=1) as const,
            tc.tile_pool(name="work", bufs=2) as work,
            tc.tile_pool(name="psum", bufs=1, space="PSUM") as psum,
            tc.tile_pool(name="dram", bufs=1, space="DRAM") as dram,
        ):
            ident = const.tile([128, 128], f16)
            make_identity(nc, ident)
            bqs = const.tile([128, HEADS], f32)
            bks = const.tile([128, 1], f32)
            bvs = const.tile([128, 1], f32)
            bob = const.tile([128, D], f16)

            wk_sb = const.tile([128, NKS, 128], f16)
            wv_sb = const.tile([128, NKS, 128], f16)
            # Wo by head; first NQ head-slots double as x_own (dead by o_proj)
            wo_sb = const.tile([128, HEADS, NNB * TCH], f16)
            x_own = wo_sb[:, 0:NQ, :]

            kt = const.tile([128, GROUPS, T], f16)  # gathered K^T
            v_sb = const.tile([128, GROUPS, NSB, 132], f16)  # gathered V + ones
            qt = const.tile([128, HEADS, TCH], f16)  # own-chunk Q^T
            at = const.tile([128, HEADS, 4, 128], f16)  # own-chunk A^T
            nc.vector.memset(v_sb[:, :, :, 128:129], 1.0)

            # initial loads, quad-interleaved so chunk 0 can start early
            for q in range(NQ):
                nc.scalar.dma_start(wk_sb[:, q * 4 : (q + 1) * 4, :],
                                    wk_d[:, q * 512 : (q + 1) * 512])
                nc.scalar.dma_start(wv_sb[:, q * 4 : (q + 1) * 4, :],
                                    wv_d[:, q * 512 : (q + 1) * 512])
                nc.scalar.dma_start(x_own[:, q, :],
                                    xcb_d[q * 128 : (q + 1) * 128, :])
            nc.scalar.dma_start(bks[:], bks_d[:])
            nc.scalar.dma_start(bvs[:], bvs_d[:])
            nc.scalar.dma_start(bqs[:], bqs_d[:])

            # wq stream on the scalar HWDGE queue
            wq_tiles = {}

            def issue_wq(h):
                if h >= HEADS or h in wq_tiles:
                    return
                wqh = work.tile([128, NKS * 128], f16, tag="wq", bufs=NWQB,
                                name="wqh", uniquify=True)
                nc.scalar.dma_start(
                    wqh[:], wq_d[:, h * NKS * 128 : (h + 1) * NKS * 128])
                wq_tiles[h] = wqh

            issue_wq(0)
            issue_wq(1)

            # ---- phase 1: K/V projection for own group, all T ----
            kvl = {}
            kvg = {}
            for a in range(AG_SPLIT):
                kvl[a] = dram.tile([256, CPA * TCH], f16, tag=f"kvl{a}",
                                   name=f"kvl{a}")
                kvg[a] = dram.tile([4 * 256, CPA * TCH], f16, tag=f"kvg{a}",
                                   name=f"kvg{a}")

            for c in range(NTCH):
                kacc = psum.tile([128, TCH], f32, tag="half", bufs=2, name="kacc")
                vacc = psum.tile([128, TCH], f32, tag="half", bufs=2, name="vacc")
                for q in range(NQ):
                    if c == 0:
                        x4 = x_own[:, q, :]
                    else:
                        x4t = work.tile([128, 4 * TCH], f16, tag="xs", bufs=3,
                                        name="x4t")
                        nc.sync.dma_start(
                            x4t[:],
                            xcb_d[(c * NQ + q) * 128 : (c * NQ + q + 1) * 128, :],
                        )
                        x4 = x4t[:]
                    for k2 in range(4):
                        ks = q * 4 + k2
                        xb = x4[:, k2 * TCH : (k2 + 1) * TCH]
                        nc.tensor.matmul(
                            kacc[:], wk_sb[:, ks, :], xb,
                            start=(ks == 0), stop=(ks == NKS - 1),
                        )
                        nc.tensor.matmul(
                            vacc[:], wv_sb[:, ks, :], xb,
                            start=(ks == 0), stop=(ks == NKS - 1),
                        )
                a, cc = c // CPA, c % CPA
                ktc = work.tile([128, TCH], f16, tag="ktc", bufs=2, name="ktc")
                nc.vector.tensor_scalar_add(ktc[:], kacc[:], bks[:, 0:1])
                nc.scalar.dma_start(kvl[a][0:128, cc * TCH : (cc + 1) * TCH],
                                    ktc[:])
                vtc = work.tile([128, TCH], f16, tag="vtc", bufs=2, name="vtc")
                nc.vector.tensor_scalar_add(vtc[:], vacc[:], bvs[:, 0:1])
                # PE transpose per 128-block (an XBAR dma-transpose here
                # head-of-line blocks the scalar queue on chunk deps)
                vn = work.tile([128, 4, 128], f16, tag="vt", bufs=2, name="vn")
                for sb in range(4):
                    tp = psum.tile([128, 128], f16, tag="big", bufs=3, name="tp")
                    nc.tensor.transpose(tp[:], vtc[:, sb * 128 : (sb + 1) * 128],
                                        ident[:])
                    nc.vector.tensor_copy(vn[:, sb, :], tp[:])
                nc.scalar.dma_start(kvl[a][128:256, cc * TCH : (cc + 1) * TCH],
                                    vn[:])
                # cap at tile NWQB-1: deeper tiles would wait on Q-proj
                # consumption and head-of-line block the scalar queue
                issue_wq(min(2 + 2 * c, NWQB - 1))
                issue_wq(min(3 + 2 * c, NWQB - 1))
                if cc == CPA - 1:
                    nc.gpsimd.collective_compute(
                        "AllGather", mybir.AluOpType.bypass,
                        replica_groups=groups,
                        ins=[kvl[a][:]], outs=[kvg[a][:]],
                    )

            # ---- phase 2: Q projection (own chunk, all heads); overlaps AG ----
            for h in range(HEADS):
                wqh = wq_tiles[h]
                qacc = psum.tile([128, TCH], f32, tag="half", bufs=2, name="qacc")
                for ks in range(NKS):
                    nc.tensor.matmul(
                        qacc[:], wqh[:, ks * 128 : (ks + 1) * 128],
                        x_own[:, ks // 4, (ks % 4) * TCH : (ks % 4 + 1) * TCH],
                        start=(ks == 0), stop=(ks == NKS - 1),
                    )
                nc.vector.tensor_scalar(
                    qt[:, h, :], qacc[:], SCALE, bqs[:, h : h + 1],
                    op0=Mult, op1=Add,
                )
                issue_wq(h + NWQB)

            nc.scalar.dma_start(bob[:], bob_d[:])

            # unpack gathered K^T / V into SBUF; groups 0/1 on sync run as
            # soon as each AG lands, groups 2/3 ride scalar behind the wq
            # stream (resolved well before their attention turn)
            for g in range(GROUPS):
                eng = nc.sync if g < 2 else nc.scalar
                for a in range(AG_SPLIT):
                    eng.dma_start(
                        kt[:, g, a * CPA * TCH : (a + 1) * CPA * TCH],
                        kvg[a][g * 256 : g * 256 + 128, :],
                    )
                    eng.dma_start(
                        v_sb[:, g, a * CPA * 4 : (a + 1) * CPA * 4, 0:128],
                        kvg[a][g * 256 + 128 : (g + 1) * 256, :],
                    )

            # ---- phase 3: attention for own chunk, all heads ----
            # Wo head-tiles stream on gpsimd during attention; each DMA is
            # gated on a token DVE-write fed by the head's normalize output
            # (dep-free DMAs would otherwise be scheduled at t=0 and fight
            # the x/wq streams for HBM). Heads 0..3 also wait out Q-proj's
            # x_own reads.
            wo_order = list(range(NQ, HEADS)) + list(range(NQ))
            for g in range(GROUPS):
                for hh in range(M):
                    h = g * M + hh
                    opk01 = psum.tile([128, 258], f32, tag="half", bufs=2,
                                      name="opk01")
                    opk23 = psum.tile([128, 258], f32, tag="half", bufs=2,
                                      name="opk23")
                    opks = [(opk01, 0), (opk01, 129), (opk23, 0), (opk23, 129)]
                    for sp in range(NSB // 2):
                        sps2 = psum.tile([128, 2 * TCH], f32, tag="big", bufs=3,
                                         name="sps2")
                        for j in range(2):
                            s = sp * 2 + j
                            nc.tensor.matmul(
                                sps2[:, j * TCH : (j + 1) * TCH],
                                kt[:, g, s * 128 : (s + 1) * 128], qt[:, h, :],
                                start=True, stop=True,
                            )
                        p2 = work.tile([128, 2 * TCH], f16, tag="p", bufs=4,
                                       name="p2")
                        nc.scalar.activation(p2[:], sps2[:], Exp)
                        for j in range(2):
                            s = sp * 2 + j
                            for tb in range(4):
                                opk, off = opks[tb]
                                # start=True clears the WHOLE PSUM bank's
                                # has_written bits, so only the first group
                                # in each shared bank may assert it; the
                                # off=129 group's s=0 matmul writes fresh
                                # (per-element has_written=0) with start=False
                                nc.tensor.matmul(
                                    opk[:, off : off + 129],
                                    p2[:, j * TCH + tb * 128
                                       : j * TCH + (tb + 1) * 128],
                                    v_sb[:, g, s, 0:129],
                                    start=(s == 0 and off == 0),
                                    stop=(s == NSB - 1),
                                    skip_group_check=(off != 0),
                                )
                    o_sb = work.tile([128, TCH], f16, tag="osb", bufs=2,
                                     name="o_sb")
                    for tb in range(4):
                        opk, off = opks[tb]
                        rcp = work.tile([128, 1], f32, tag="rcp", bufs=4,
                                        name="rcp")
                        nc.vector.reciprocal(rcp[:], opk[:, off + 128 : off + 129])
                        nc.vector.tensor_scalar_mul(
                            o_sb[:, tb * 128 : (tb + 1) * 128],
                            opk[:, off : off + 128], rcp[:])
                    # at[:, h, tb, :] = o_sb[:, tb*128:+128].T via XBAR
                    nc.sync.dma_start_transpose(at[:, h], o_sb[:])
                    # token gate + Wo head-tile stream (see comment above)
                    ws = wo_order[h]
                    nc.vector.tensor_copy(wo_sb[:, ws, 0:1], o_sb[:, 0:1])
                    nc.gpsimd.dma_start(
                        wo_sb[:, ws, :],
                        wo_d[:, ws * NNB * TCH : (ws + 1) * NNB * TCH],
                    )

            # ---- phase 4: o_proj for own chunk, full D ----
            for tb in range(4):
                pp01 = psum.tile([128, 2 * TCH], f32, tag="big", bufs=3,
                                 name="pp01")
                pp23 = psum.tile([128, 2 * TCH], f32, tag="big", bufs=3,
                                 name="pp23")
                pps = [(pp01, 0), (pp01, TCH), (pp23, 0), (pp23, TCH)]
                for h in range(HEADS):
                    for nb in range(NNB):
                        pp, off = pps[nb]
                        nc.tensor.matmul(
                            pp[:, off : off + TCH],
                            at[:, h, tb, :],
                            wo_sb[:, h, nb * TCH : (nb + 1) * TCH],
                            start=(h == 0), stop=(h == HEADS - 1),
                        )
                for nb in range(NNB):
                    pp, off = pps[nb]
                    ob = work.tile([128, TCH], f16, tag="ob", bufs=4, name="ob")
                    nc.vector.scalar_tensor_tensor(
                        ob[:], pp[:, off : off + TCH], 1.0,
                        bob[:, nb * TCH : (nb + 1) * TCH],
                        op0=Mult, op1=Add,
                    )
                    nc.sync.dma_start(
                        out_d[tb * 128 : (tb + 1) * 128, nb * TCH : (nb + 1) * TCH],
                        ob[:],
                    )

    nc.compile()
    return nc


def _get_nc():
    if "nc" not in _COMPILED:
        _COMPILED["nc"] = _build()
    return _COMPILED["nc"]


def kernel(x, Wq, bq, Wk, bk, Wv, bv, Wo, bo):
    from concourse.bass_utils import run_bass_kernel_spmd

    x = np.asarray(x, np.float32)
    Wq = np.asarray(Wq, np.float32)
    Wk = np.asarray(Wk, np.float32)
    Wv = np.asarray(Wv, np.float32)
    Wo = np.asarray(Wo, np.float32)
    bq = np.asarray(bq, np.float32)
    bk = np.asarray(bk, np.float32)
    bv = np.asarray(bv, np.float32)
    bo = np.asarray(bo, np.float32)

    nc = _get_nc()

    # shared across cores
    wq_h = np.ascontiguousarray(
        Wq.reshape(NKS, 128, HEADS, 128).transpose(1, 2, 0, 3).reshape(128, -1)
    ).astype(np.float16)
    # by-head o_proj layout [128, h, nb, 512]
    wo_h = np.ascontiguousarray(
        Wo.reshape(HEADS, 128, NNB, TCH).transpose(1, 0, 2, 3).reshape(128, -1)
    ).astype(np.float16)
    bqs_h = np.ascontiguousarray((bq * SCALE).reshape(HEADS, 128).T)
    bob_h = np.ascontiguousarray(np.broadcast_to(bo.astype(np.float16), (128, D)))
    # x^T per batch, pre-blocked into (chunk, quad) [128, 2048] row-blocks
    xq16 = []
    for b in range(B):
        xTb = x[b].T.astype(np.float16)  # [D, T]
        blocks = xTb.reshape(NKS, 128, NTCH, TCH).transpose(2, 0, 1, 3)
        # [chunk, ks, 128, TCH] -> quads: [chunk, quad, 128, 4*TCH]
        blocks = blocks.reshape(NTCH, NQ, 4, 128, TCH).transpose(0, 1, 3, 2, 4)
        xq16.append(np.ascontiguousarray(blocks.reshape(NTCH, NQ * 128, 4 * TCH)))

    wk_g, wv_g, bks_g, bvs_g = [], [], [], []
    for g in range(GROUPS):
        wk_g.append(
            np.ascontiguousarray(
                Wk[:, g * HD : (g + 1) * HD].reshape(NKS, 128, HD)
                .transpose(1, 0, 2).reshape(128, -1)
            ).astype(np.float16)
        )
        wv_g.append(
            np.ascontiguousarray(
                Wv[:, g * HD : (g + 1) * HD].reshape(NKS, 128, HD)
                .transpose(1, 0, 2).reshape(128, -1)
            ).astype(np.float16)
        )
        bks_g.append(np.ascontiguousarray(bk[g * HD : (g + 1) * HD].reshape(1, HD).T))
        bvs_g.append(np.ascontiguousarray(bv[g * HD : (g + 1) * HD].reshape(1, HD).T))

    in_maps = []
    for c in range(N_CORES):
        b, r = c // 4, c % 4
        order = [r] + [i for i in range(NTCH) if i != r]
        xcb = np.concatenate([xq16[b][s] for s in order], axis=0)
        in_maps.append(
            {
                "xcb": np.ascontiguousarray(xcb),
                "wq": wq_h,
                "wk": wk_g[r],
                "wv": wv_g[r],
                "wo": wo_h,
                "bqs": bqs_h,
                "bks": bks_g[r],
                "bvs": bvs_g[r],
                "bob": bob_h,
            }
        )

    res = run_bass_kernel_spmd(nc, in_maps, list(range(N_CORES)))
    _COMPILED["last_res"] = res

    out = np.empty((B, T, D), np.float32)
    for b in range(B):
        for r in range(NTCH):
            out[b, r * TCH : (r + 1) * TCH, :] = (
                res.results[4 * b + r]["out"].astype(np.float32)
            )
    return out


# revision 15
# speedup vs baseline: 1.2031x; 1.0626x over previous
"""GroupedQueryAttention on 8 Trainium2 NeuronCores (v3).

Sharding: core c = 4*b + r handles batch b (of 2) and token chunk r (512
of 2048 tokens) for Q/attention/o_proj over ALL 16 heads. K/V projections
are sharded by KV group: core r computes group g=r's K/V for all T, then
chunk-split AllGathers across each batch's 4 cores make every core
independent for the rest of the kernel -- no output collective.

v3 deltas over the 492us baseline:
  - phase 1 runs chunks in payload order [own,1,2,3] and pushes each
    chunk's K/V payload immediately; the AllGather is split in
    AG_SPLIT pieces issued as soon as their chunks are done, so the
    collective overlaps phase 1 + Q-proj instead of trailing them.
  - the 8 MB wq stream lives on the scalar HWDGE queue (paced 1-2 tiles
    per phase-1 chunk, 7-deep) so Q-proj never starves: the baseline
    lost ~50us to wq waiting behind x/unpack traffic on sync.
  - every PE transpose (16 V blocks + 64 attention outputs) is replaced
    by an XBAR dma_start_transpose ([128,512] -> [128,4,128] blocked
    transpose, ~0.5us on a DMA queue), freeing ~14us of PE and the DVE
    copies that drained them.
  - o_proj is h-outer (each at-block stationary feeds 4 matmuls), bias
    is a DVE add fused into the PSUM drain (identity-matmul bias gone),
    and out is written fp16 (host upcasts).
  - Wo lives in one 8 MB SBUF buffer whose first quarter aliases x_own
    (dead after Q-proj); Wo head-tiles 4..15 stream during attention,
    0..3 after Q-proj reads retire. Scalar queue carries no DMA during
    attention so exp (the 138us ACT-bound softmax) never waits.
  - PSUM: tag "big" = 3 x [128,1024] (scores 3-deep, o_proj nb-pairs),
    tag "half" = 2 x [128,512]-sized slots holding K/V/Q accumulators
    and the paired A@V accumulators [128,258] (2 x (128 out + 1 denom)).

All matmuls fp16 (1 PE cycle/row) with fp32 PSUM accumulation. Layouts
avoid transposing the big P matrix: projections produce Q^T/K^T/V^T
directly; scores are S^T = (K^T block).T @ Q^T; exp(S^T) = P^T feeds
A@V as the stationary; V carries a ones-column so the softmax
denominator falls out of the A@V matmul for free.
"""

import math
import sys

import numpy as np

sys.path.insert(0, "/opt/trn_rl_repo")

B = 2
T = 2048
D = 2048
HEADS = 16
GROUPS = 4
HD = 128  # head dim
M = HEADS // GROUPS  # heads per group = 4
SCALE = 1.0 / math.sqrt(HD)
N_CORES = 8
TCH = 512  # token chunk per core
NTCH = T // TCH  # 4
NSB = T // 128  # 16 key blocks
NKS = D // 128  # 16 contraction steps for projections
NQ = NKS // 4  # 4 quad blocks for the x stream
NNB = D // TCH  # 4 o_proj output column blocks

NWQB = 6  # wq stream depth
AG_SPLIT = 2  # number of chunk-split AllGathers
CPA = NTCH // AG_SPLIT  # chunks per AllGather

_COMPILED = {}


def _build():
    import concourse.bass as bass
    import concourse.mybir as mybir
    import concourse.tile as tile
    from concourse import bacc
    from concourse.masks import make_identity

    f16 = mybir.dt.float16
    f32 = mybir.dt.float32
    Exp = mybir.ActivationFunctionType.Exp
    Mult = mybir.AluOpType.mult
    Add = mybir.AluOpType.add

    nc = bacc.Bacc("TRN2", target_bir_lowering=False, num_devices=N_CORES)

    # x^T as (slot, quad) row-blocks of [128, 2048]; slot 0 = own chunk
    xcb_d = nc.declare_dram_parameter("xcb", [NTCH * NQ * 128, 4 * TCH], f16,
                                      isOutput=False)
    wq_d = nc.declare_dram_parameter("wq", [128, HEADS * NKS * 128], f16,
                                     isOutput=False)
    wk_d = nc.declare_dram_parameter("wk", [128, NKS * 128], f16, isOutput=False)
    wv_d = nc.declare_dram_parameter("wv", [128, NKS * 128], f16, isOutput=False)
    # by-head layout: [128, h, nb, 512]
    wo_d = nc.declare_dram_parameter("wo", [128, HEADS * NNB * TCH], f16,
                                     isOutput=False)
    bqs_d = nc.declare_dram_parameter("bqs", [128, HEADS], f32, isOutput=False)
    bks_d = nc.declare_dram_parameter("bks", [128, 1], f32, isOutput=False)
    bvs_d = nc.declare_dram_parameter("bvs", [128, 1], f32, isOutput=False)
    bob_d = nc.declare_dram_parameter("bob", [128, D], f16, isOutput=False)
    out_d = nc.declare_dram_parameter("out", [TCH, D], f16, isOutput=True)

    groups = [[0, 1, 2, 3], [4, 5, 6, 7]]

    with tile.TileContext(nc) as tc:
        with (
            tc.tile_pool(name="const", bufs=1) as const,
            tc.tile_pool(name="work", bufs=2) as work,
            tc.tile_pool(name="psum", bufs=1, space="PSUM") as psum,
            tc.tile_pool(name="dram", bufs=1, space="DRAM") as dram,
        ):
            ident = const.tile([128, 128], f16)
            make_identity(nc, ident)
            bqs = const.tile([128, HEADS], f32)
            bks = const.tile([128, 1], f32)
            bvs = const.tile([128, 1], f32)
            bob = const.tile([128, D], f16)

            wk_sb = const.tile([128, NKS, 128], f16)
            wv_sb = const.tile([128, NKS, 128], f16)
            # Wo by head; first NQ head-slots double as x_own (dead by o_proj)
            wo_sb = const.tile([128, HEADS, NNB * TCH], f16)
            x_own = wo_sb[:, 0:NQ, :]

            kt = const.tile([128, GROUPS, T], f16)  # gathered K^T
            v_sb = const.tile([128, GROUPS, NSB, 132], f16)  # gathered V + ones
            qt = const.tile([128, HEADS, TCH], f16)  # own-chunk Q^T
            at = const.tile([128, HEADS, 4, 128], f16)  # own-chunk A^T
            nc.vector.memset(v_sb[:, :, :, 128:129], 1.0)

            # initial loads, quad-interleaved so chunk 0 can start early;
            # x_own rides sync so it doesn't serialize behind wk/wv
            for q in range(NQ):
                nc.scalar.dma_start(wk_sb[:, q * 4 : (q + 1) * 4, :],
                                    wk_d[:, q * 512 : (q + 1) * 512])
                nc.scalar.dma_start(wv_sb[:, q * 4 : (q + 1) * 4, :],
                                    wv_d[:, q * 512 : (q + 1) * 512])
                nc.sync.dma_start(x_own[:, q, :],
                                  xcb_d[q * 128 : (q + 1) * 128, :])
            nc.scalar.dma_start(bks[:], bks_d[:])
            nc.scalar.dma_start(bvs[:], bvs_d[:])
            nc.scalar.dma_start(bqs[:], bqs_d[:])

            # wq stream on the scalar HWDGE queue
            wq_tiles = {}

            def issue_wq(h):
                if h >= HEADS or h in wq_tiles:
                    return
                wqh = work.tile([128, NKS * 128], f16, tag="wq", bufs=NWQB,
                                name="wqh", uniquify=True)
                eng = nc.scalar if h % 2 == 0 else nc.sync
                eng.dma_start(
                    wqh[:], wq_d[:, h * NKS * 128 : (h + 1) * NKS * 128])
                wq_tiles[h] = wqh

            issue_wq(0)
            issue_wq(1)

            # ---- phase 1: K/V projection for own group, all T ----
            kvl = {}
            kvg = {}
            for a in range(AG_SPLIT):
                kvl[a] = dram.tile([256, CPA * TCH], f16, tag=f"kvl{a}",
                                   name=f"kvl{a}")
                kvg[a] = dram.tile([4 * 256, CPA * TCH], f16, tag=f"kvg{a}",
                                   name=f"kvg{a}")

            for c in range(NTCH):
                kacc = psum.tile([128, TCH], f32, tag="half", bufs=2, name="kacc")
                vacc = psum.tile([128, TCH], f32, tag="half", bufs=2, name="vacc")
                for q in range(NQ):
                    if c == 0:
                        x4 = x_own[:, q, :]
                    else:
                        x4t = work.tile([128, 4 * TCH], f16, tag="xs", bufs=3,
                                        name="x4t")
                        nc.sync.dma_start(
                            x4t[:],
                            xcb_d[(c * NQ + q) * 128 : (c * NQ + q + 1) * 128, :],
                        )
                        x4 = x4t[:]
                    for k2 in range(4):
                        ks = q * 4 + k2
                        xb = x4[:, k2 * TCH : (k2 + 1) * TCH]
                        nc.tensor.matmul(
                            kacc[:], wk_sb[:, ks, :], xb,
                            start=(ks == 0), stop=(ks == NKS - 1),
                        )
                        nc.tensor.matmul(
                            vacc[:], wv_sb[:, ks, :], xb,
                            start=(ks == 0), stop=(ks == NKS - 1),
                        )
                a, cc = c // CPA, c % CPA
                ktc = work.tile([128, TCH], f16, tag="ktc", bufs=2, name="ktc")
                nc.vector.tensor_scalar_add(ktc[:], kacc[:], bks[:, 0:1])
                nc.scalar.dma_start(kvl[a][0:128, cc * TCH : (cc + 1) * TCH],
                                    ktc[:])
                vtc = work.tile([128, TCH], f16, tag="vtc", bufs=2, name="vtc")
                nc.vector.tensor_scalar_add(vtc[:], vacc[:], bvs[:, 0:1])
                # PE transpose per 128-block (an XBAR dma-transpose here
                # head-of-line blocks the scalar queue on chunk deps)
                vn = work.tile([128, 4, 128], f16, tag="vt", bufs=2, name="vn")
                for sb in range(4):
                    tp = psum.tile([128, 128], f16, tag="big", bufs=2, name="tp")
                    nc.tensor.transpose(tp[:], vtc[:, sb * 128 : (sb + 1) * 128],
                                        ident[:])
                    nc.vector.tensor_copy(vn[:, sb, :], tp[:])
                nc.scalar.dma_start(kvl[a][128:256, cc * TCH : (cc + 1) * TCH],
                                    vn[:])
                # cap at tile NWQB-1: deeper tiles would wait on Q-proj
                # consumption and head-of-line block the scalar queue
                issue_wq(min(2 + 2 * c, NWQB - 1))
                issue_wq(min(3 + 2 * c, NWQB - 1))
                if cc == CPA - 1:
                    nc.gpsimd.collective_compute(
                        "AllGather", mybir.AluOpType.bypass,
                        replica_groups=groups,
                        ins=[kvl[a][:]], outs=[kvg[a][:]],
                    )

            nc.scalar.dma_start(bob[:], bob_d[:])

            # unpack gathered K^T / V into SBUF on the gpsimd queue (behind
            # the AG issues; nothing latency-critical queues after them)
            for g in range(GROUPS):
                for a in range(AG_SPLIT):
                    nc.gpsimd.dma_start(
                        kt[:, g, a * CPA * TCH : (a + 1) * CPA * TCH],
                        kvg[a][g * 256 : g * 256 + 128, :],
                    )
                    nc.gpsimd.dma_start(
                        v_sb[:, g, a * CPA * 4 : (a + 1) * CPA * 4, 0:128],
                        kvg[a][g * 256 + 128 : (g + 1) * 256, :],
                    )

            # ---- phases 2+3 interleaved: Q-proj for group g's heads, then
            # attention for group g (its exp-wait gaps absorb the next
            # group's Q-proj matmuls) ----
            def qproj_head(h):
                wqh = wq_tiles[h]
                qacc = psum.tile([128, TCH], f32, tag="half", bufs=2, name="qacc")
                for ks in range(NKS):
                    nc.tensor.matmul(
                        qacc[:], wqh[:, ks * 128 : (ks + 1) * 128],
                        x_own[:, ks // 4, (ks % 4) * TCH : (ks % 4 + 1) * TCH],
                        start=(ks == 0), stop=(ks == NKS - 1),
                    )
                nc.vector.tensor_scalar(
                    qt[:, h, :], qacc[:], SCALE, bqs[:, h : h + 1],
                    op0=Mult, op1=Add,
                )
                issue_wq(h + NWQB)

            # Wo head-tiles stream on gpsimd during attention; each DMA is
            # gated on a token DVE-write fed by the head's normalize output
            # (dep-free DMAs would otherwise be scheduled at t=0 and fight
            # the x/wq streams for HBM). Heads 0..3 also wait out Q-proj's
            # x_own reads.
            wo_order = list(range(NQ, HEADS)) + list(range(NQ))
            for g in range(GROUPS):
                for hh in range(M):
                    qproj_head(g * M + hh)
                for hh in range(M):
                    h = g * M + hh
                    opk01 = psum.tile([128, 258], f32, tag="opk", bufs=2,
                                      name="opk01")
                    opk23 = psum.tile([128, 258], f32, tag="opk", bufs=2,
                                      name="opk23")
                    opks = [(opk01, 0), (opk01, 129), (opk23, 0), (opk23, 129)]
                    for sp in range(NSB // 2):
                        sps2 = psum.tile([128, 2 * TCH], f32, tag="big", bufs=2,
                                         name="sps2")
                        for j in range(2):
                            s = sp * 2 + j
                            nc.tensor.matmul(
                                sps2[:, j * TCH : (j + 1) * TCH],
                                kt[:, g, s * 128 : (s + 1) * 128], qt[:, h, :],
                                start=True, stop=True,
                            )
                        p2 = work.tile([128, 2 * TCH], f16, tag="p", bufs=4,
                                       name="p2")
                        nc.scalar.activation(p2[:], sps2[:], Exp)
                        for j in range(2):
                            s = sp * 2 + j
                            for tb in range(4):
                                opk, off = opks[tb]
                                # start=True clears the WHOLE PSUM bank's
                                # has_written bits, so only the first group
                                # in each shared bank may assert it; the
                                # off=129 group's s=0 matmul writes fresh
                                # (per-element has_written=0) with start=False
                                nc.tensor.matmul(
                                    opk[:, off : off + 129],
                                    p2[:, j * TCH + tb * 128
                                       : j * TCH + (tb + 1) * 128],
                                    v_sb[:, g, s, 0:129],
                                    start=(s == 0 and off == 0),
                                    stop=(s == NSB - 1),
                                    skip_group_check=(off != 0),
                                )
                    o_sb = work.tile([128, TCH], f16, tag="osb", bufs=2,
                                     name="o_sb")
                    for tb in range(4):
                        opk, off = opks[tb]
                        rcp = work.tile([128, 1], f32, tag="rcp", bufs=4,
                                        name="rcp")
                        nc.vector.reciprocal(rcp[:], opk[:, off + 128 : off + 129])
                        nc.vector.tensor_scalar_mul(
                            o_sb[:, tb * 128 : (tb + 1) * 128],
                            opk[:, off : off + 128], rcp[:])
                    # at[:, h, tb, :] = o_sb[:, tb*128:+128].T via XBAR
                    nc.sync.dma_start_transpose(at[:, h], o_sb[:])
                    # token gate + Wo head-tile stream (see comment above)
                    ws = wo_order[h]
                    nc.vector.tensor_copy(wo_sb[:, ws, 0:1], o_sb[:, 0:1])
                    nc.gpsimd.dma_start(
                        wo_sb[:, ws, :],
                        wo_d[:, ws * NNB * TCH : (ws + 1) * NNB * TCH],
                    )

            # ---- phase 4: o_proj for own chunk, full D ----
            for tb in range(4):
                pp01 = psum.tile([128, 2 * TCH], f32, tag="big", bufs=2,
                                 name="pp01")
                pp23 = psum.tile([128, 2 * TCH], f32, tag="big", bufs=2,
                                 name="pp23")
                pps = [(pp01, 0), (pp01, TCH), (pp23, 0), (pp23, TCH)]
                for h in range(HEADS):
                    for nb in range(NNB):
                        pp, off = pps[nb]
                        nc.tensor.matmul(
                            pp[:, off : off + TCH],
                            at[:, h, tb, :],
                            wo_sb[:, h, nb * TCH : (nb + 1) * TCH],
                            start=(h == 0), stop=(h == HEADS - 1),
                        )
                for nb in range(NNB):
                    pp, off = pps[nb]
                    ob = work.tile([128, TCH], f16, tag="ob", bufs=4, name="ob")
                    nc.vector.scalar_tensor_tensor(
                        ob[:], pp[:, off : off + TCH], 1.0,
                        bob[:, nb * TCH : (nb + 1) * TCH],
                        op0=Mult, op1=Add,
                    )
                    nc.sync.dma_start(
                        out_d[tb * 128 : (tb + 1) * 128, nb * TCH : (nb + 1) * TCH],
                        ob[:],
                    )

    nc.compile()
    return nc


def _get_nc():
    if "nc" not in _COMPILED:
        _COMPILED["nc"] = _build()
    return _COMPILED["nc"]


def kernel(x, Wq, bq, Wk, bk, Wv, bv, Wo, bo):
    from concourse.bass_utils import run_bass_kernel_spmd

    x = np.asarray(x, np.float32)
    Wq = np.asarray(Wq, np.float32)
    Wk = np.asarray(Wk, np.float32)
    Wv = np.asarray(Wv, np.float32)
    Wo = np.asarray(Wo, np.float32)
    bq = np.asarray(bq, np.float32)
    bk = np.asarray(bk, np.float32)
    bv = np.asarray(bv, np.float32)
    bo = np.asarray(bo, np.float32)

    nc = _get_nc()

    # shared across cores
    wq_h = np.ascontiguousarray(
        Wq.reshape(NKS, 128, HEADS, 128).transpose(1, 2, 0, 3).reshape(128, -1)
    ).astype(np.float16)
    # by-head o_proj layout [128, h, nb, 512]
    wo_h = np.ascontiguousarray(
        Wo.reshape(HEADS, 128, NNB, TCH).transpose(1, 0, 2, 3).reshape(128, -1)
    ).astype(np.float16)
    bqs_h = np.ascontiguousarray((bq * SCALE).reshape(HEADS, 128).T)
    bob_h = np.ascontiguousarray(np.broadcast_to(bo.astype(np.float16), (128, D)))
    # x^T per batch, pre-blocked into (chunk, quad) [128, 2048] row-blocks
    xq16 = []
    for b in range(B):
        xTb = x[b].T.astype(np.float16)  # [D, T]
        blocks = xTb.reshape(NKS, 128, NTCH, TCH).transpose(2, 0, 1, 3)
        # [chunk, ks, 128, TCH] -> quads: [chunk, quad, 128, 4*TCH]
        blocks = blocks.reshape(NTCH, NQ, 4, 128, TCH).transpose(0, 1, 3, 2, 4)
        xq16.append(np.ascontiguousarray(blocks.reshape(NTCH, NQ * 128, 4 * TCH)))

    wk_g, wv_g, bks_g, bvs_g = [], [], [], []
    for g in range(GROUPS):
        wk_g.append(
            np.ascontiguousarray(
                Wk[:, g * HD : (g + 1) * HD].reshape(NKS, 128, HD)
                .transpose(1, 0, 2).reshape(128, -1)
            ).astype(np.float16)
        )
        wv_g.append(
            np.ascontiguousarray(
                Wv[:, g * HD : (g + 1) * HD].reshape(NKS, 128, HD)
                .transpose(1, 0, 2).reshape(128, -1)
            ).astype(np.float16)
        )
        bks_g.append(np.ascontiguousarray(bk[g * HD : (g + 1) * HD].reshape(1, HD).T))
        bvs_g.append(np.ascontiguousarray(bv[g * HD : (g + 1) * HD].reshape(1, HD).T))

    in_maps = []
    for c in range(N_CORES):
        b, r = c // 4, c % 4
        order = [r] + [i for i in range(NTCH) if i != r]
        xcb = np.concatenate([xq16[b][s] for s in order], axis=0)
        in_maps.append(
            {
                "xcb": np.ascontiguousarray(xcb),
                "wq": wq_h,
                "wk": wk_g[r],
                "wv": wv_g[r],
                "wo": wo_h,
                "bqs": bqs_h,
                "bks": bks_g[r],
                "bvs": bvs_g[r],
                "bob": bob_h,
            }
        )

    res = run_bass_kernel_spmd(nc, in_maps, list(range(N_CORES)))
    _COMPILED["last_res"] = res

    out = np.empty((B, T, D), np.float32)
    for b in range(B):
        for r in range(NTCH):
            out[b, r * TCH : (r + 1) * TCH, :] = (
                res.results[4 * b + r]["out"].astype(np.float32)
            )
    return out
